# revision 52
# baseline (speedup 1.0000x reference)
"""Trainium2 Bass kernel for the Antenna message-generation MLP.

Reference computation (per batch b, RF-chain r, antenna u):
    x[b,r,u,:48] = concat(F[b,:,r], sum_u C[b,u,r,:], H[b,u,8r:8r+8], H[b,u,64+8r:64+8r+8])
    out[b,r,u,:] = tanh(relu(relu(x@W1+b1)@W2+b2)@W3+b3)

Strategy: pure data parallelism over the batch dim across 8 NeuronCores
(256 batches/core).  Each core processes 16 chunks of 1024 rows.  The
48-dim input vectors are packed TWO per 128 partitions (K=64 packs):
partitions [0:64] hold rows 0-511 of the chunk, [64:128] rows 512-1023,
with bands [0:16]=c (u-summed C), [16]=ones (b1 fold), [32:48]=h,
[48:64]=F within each pack.  L1 then runs as pairs of row-tiled
matmuls (tile_position (0,0)/(64,0)) that execute concurrently on
disjoint PE row groups - half the streaming cycles of an unpacked L1.
Band placement respects engine partition-base rules: DVE writes c/h at
quadrant-aligned bases, F is DMA'd (any base) to partitions [48:64] /
[112:128] and u-broadcast in place by GpSimd (which works on its native
16-partition slices), the ones row comes from a host-built constant.
L2 keeps features-on-partitions; its PSUM pairs both row packs per
output block so the relu+bias activation runs 1024 wide and
consecutive matmuls share stationary weights.  L3 uses the fp16 a2
blocks as the stationary operand so the output lands row-major
[rows x 16] in PSUM; b3 (free-dim-varying) is added on DVE from a
host-broadcast constant, then a [128 x 64] tanh and a direct store.
All matmul operands are fp16 (FWL fast weight loads); weights are
packed/cast to their on-chip layouts on the host, so the device
prologue is a handful of clean DMAs and the first matmul starts as
soon as chunk 0's gather lands.
"""

import sys
import types

import numpy as np

# This image's `antenv` lacks `axon_hooks`; bass_utils imports it when
# BASS_TRACE is set.  Register a no-op stand-in so tracing degrades
# gracefully instead of crashing (real hook installed by test harness).
try:
    import antenv.axon_hooks  # noqa: F401
except ImportError:
    import antenv

    _m = types.ModuleType("antenv.axon_hooks")
    _m._hook = None
    _m.set_axon_ntff_profile_hook = lambda h: setattr(_m, "_hook", h)
    _m.get_axon_ntff_profile_hook = lambda: _m._hook
    sys.modules["antenv.axon_hooks"] = _m
    antenv.axon_hooks = _m

import concourse.bacc as bacc
import concourse.mybir as mybir
import concourse.tile as tile
from concourse.bass_utils import run_bass_kernel_spmd

F32 = mybir.dt.float32
F16 = mybir.dt.float16

N_CORES = 8
B_FULL = 2048
B_SH = B_FULL // N_CORES    # 256 batches per core
U = 8
R = 8
M = 16
FDIM = 16
H1 = 512
H2 = 512

BG = 16                     # batches per build chunk
G = B_SH // BG              # 16 chunks per core
ROWS_CHUNK = BG * R * U     # 1024 rows per chunk
TILE = 512                  # rows per MLP tile (one PSUM bank of fp32)

_CACHE = {}


def _build():
    nc = bacc.Bacc("TRN2", target_bir_lowering=False, debug=False)

    C_ext = nc.dram_tensor("C", [B_SH, U, R, M], F32, kind="ExternalInput")
    F_ext = nc.dram_tensor("F", [B_SH, FDIM, R], F32, kind="ExternalInput")
    H_ext = nc.dram_tensor("H", [B_SH, U, 2 * 64], F32, kind="ExternalInput")
    # host-packed weights (see run()):
    w1a_ext = nc.dram_tensor("w1a", [128, H1], F16, kind="ExternalInput")
    w1b_ext = nc.dram_tensor("w1b", [128, H1], F16, kind="ExternalInput")
    w2p_ext = nc.dram_tensor("w2p", [128, 4, H2], F16, kind="ExternalInput")
    w3p_ext = nc.dram_tensor("w3p", [128, 4, M], F16, kind="ExternalInput")
    b2p_ext = nc.dram_tensor("b2p", [128, 4], F32, kind="ExternalInput")
    b3bc_ext = nc.dram_tensor("b3bc", [128, M], F32, kind="ExternalInput")
    xinit_ext = nc.dram_tensor("xinit", [16, TILE], F16, kind="ExternalInput")
    # fp16 output (host upcasts); tanh is in [-1,1] so fp16 quantization
    # (~5e-4 relative) is far inside the tolerance.
    out_ext = nc.dram_tensor("out", [B_SH, R, U, M], F16, kind="ExternalOutput")

    out_rows = out_ext.ap().rearrange("b r u m -> (b r u) m")  # [16384, 16]

    relu = mybir.ActivationFunctionType.Relu
    tanh = mybir.ActivationFunctionType.Tanh
    axis_x = mybir.AxisListType.X
    op_add = mybir.AluOpType.add

    with tile.TileContext(nc) as tc:
        with (
            tc.tile_pool(name="consts", bufs=1) as consts,
            tc.tile_pool(name="loads", bufs=4) as loads,
            tc.tile_pool(name="acts", bufs=3) as acts,
            tc.tile_pool(name="outs", bufs=3) as outs,
            tc.tile_pool(name="p1", bufs=3, space="PSUM") as p1p,
            tc.tile_pool(name="p2", bufs=2, space="PSUM") as p2p,
            tc.tile_pool(name="psm", bufs=1, space="PSUM") as psm,
        ):
            # ---- constants (host-packed; issued on the ACT DGE queue so the
            # sync queue starts with chunk 0's inputs) ----------------------
            w1a = consts.tile([128, H1], F16, tag="w1a")
            nc.scalar.dma_start(w1a[:], w1a_ext.ap())
            w1b = consts.tile([128, H1], F16, tag="w1b")
            nc.scalar.dma_start(w1b[:], w1b_ext.ap())
            w1x = (w1a, w1b)
            w2 = consts.tile([128, 4, H2], F16)
            w3 = consts.tile([128, 4, M], F16)
            b2 = consts.tile([128, 4], F32)
            b3bc = consts.tile([128, M], F32)

            # Persistent double-buffered X^T (fp16); one-time init of the
            # ones/zero bands [16:32] / [80:96] of both packs.
            xts = []
            for i in range(2):
                xt = consts.tile([128, TILE], F16, tag=f"xt{i}")
                nc.scalar.dma_start(xt[16:32, :], xinit_ext.ap())
                nc.scalar.dma_start(xt[80:96, :], xinit_ext.ap())
                xts.append(xt)
            # Persistent double-buffered F staging.  Compute ops may only
            # start at partition 0/32/64/96, so the GpSimd u-broadcast runs
            # on the whole [32:64] / [96:128] windows; the halves under the
            # h bands are zeroed once here and the real h data overwrites
            # the junk broadcast afterwards.
            fstgs = []
            for i in range(2):
                fstg = consts.tile([128, BG // 2, R], F32, tag=f"fstg{i}")
                nc.gpsimd.memset(fstg[32:48], 0.0)
                nc.gpsimd.memset(fstg[96:112], 0.0)
                fstgs.append(fstg)

            # PE warmup: the HAM clock gate keeps the array at 1.2 GHz
            # until ~3.4us of sustained matmul activity.  Chunk 0's gather
            # takes ~15us, so the first real matmuls would run cold.  Burn
            # the idle wait on dummy matmuls (reading the initialized ones
            # band; nothing reads the results) so the real work starts at
            # 2.4 GHz.
            ps_w = p1p.tile([128, TILE], F32, tag="ps1")
            for _ in range(12):
                nc.tensor.matmul(
                    ps_w[:], w1a[:, 0:128], w1a[:, :],
                    start=True, stop=True,
                )

            def gather(g):
                b0 = g * BG
                xt = xts[g % 2]
                # ---- load chunk into r-padded 32-col blocks ---------------
                # c_pad [128 x 256]: cols r*32 + (m | 16 pad)
                # h_pad [128 x 256]: cols r*32 + (i*8+k | 16 pad)
                # per-chunk F slices first (longest dependent chain: DMA ->
                # GpSimd broadcast), feature-major, staged directly at the
                # partitions of the F bands (GpSimd broadcasts in place).
                fstg = fstgs[g % 2]
                nc.sync.dma_start(
                    fstg[48:64],
                    F_ext[b0 : b0 + 8].rearrange("b f r -> f b r"),
                )
                nc.sync.dma_start(
                    fstg[112:128],
                    F_ext[b0 + 8 : b0 + BG].rearrange("b f r -> f b r"),
                )
                c_pad = loads.tile([BG * U, 256], F32, tag="c_pad")
                nc.gpsimd.memset(c_pad[:], 0.0)
                cp_v = c_pad[:].rearrange("p (r w) -> p r w", r=R)
                nc.sync.dma_start(
                    cp_v[:, :, 0:M],
                    C_ext[b0 : b0 + BG].rearrange("b u r m -> (b u) r m"),
                )
                h_pad = loads.tile([BG * U, 256], F32, tag="h_pad")
                nc.gpsimd.memset(h_pad[:], 0.0)
                hp_v = h_pad[:].rearrange("p (r w) -> p r w", r=R)
                h_src = H_ext[b0 : b0 + BG].rearrange(
                    "b u (i r k) -> (b u) i r k", i=2, r=R
                )
                for i in range(2):
                    nc.sync.dma_start(hp_v[:, :, 8 * i : 8 * i + 8], h_src[:, i])

                if g == 0:
                    # weight loads (queue behind chunk 0's inputs; not
                    # needed until L2/L3 time)
                    nc.sync.dma_start(w2[:], w2p_ext.ap())
                    nc.sync.dma_start(w3[:], w3p_ext.ap())
                    nc.sync.dma_start(b2[:], b2p_ext.ap())
                    nc.sync.dma_start(b3bc[:], b3bc_ext.ap())

                # ---- DVE 32x32 stream transposes --------------------------
                # cT[32B+m, r*32 + b4*8 + u] = C[4B+b4, u, r, m]   (m < 16)
                # hT[32B+f, r*32 + b4*8 + u] = H-feat f of (4B+b4, u, r)
                cT = loads.tile([BG * U, 256], F32, tag="cT")
                nc.vector.transpose(cT[:], c_pad[:])
                hT = loads.tile([BG * U, 256], F32, tag="hT")
                nc.vector.transpose(hT[:], h_pad[:])

                # ---- C path: u-sum then broadcast over u ------------------
                c_red = loads.tile([BG * U, 32], F32, tag="c_red")
                nc.vector.tensor_reduce(
                    c_red[:],
                    cT[:].rearrange("p (rb u) -> p rb u", u=U),
                    axis_x, op_add,
                )
                # c_red[32B+m, r*4 + b4] = c[4B+b4, r, m]
                # pack p (p=0: rows 0-511 = batches 0-7 = bands B 0,1;
                #         p=1: rows 512-1023 = bands B 2,3)
                for p in range(2):
                    # F band first: the [32:64]-window broadcast covers the
                    # h band with junk, overwritten by the h copies below.
                    # Chunk 0 uses DVE (~4x lower latency) to shorten the
                    # startup critical path; steady state uses idle GpSimd.
                    f_eng = nc.vector if g == 0 else nc.gpsimd
                    f_eng.tensor_copy(
                        xt[64 * p + 32 : 64 * p + 64, :].rearrange(
                            "p (b r u) -> p b r u", r=R, u=U
                        ),
                        fstg[64 * p + 32 : 64 * p + 64]
                        .unsqueeze(3)
                        .broadcast_to((32, BG // 2, R, U)),
                    )
                    xt_c = xt[64 * p : 64 * p + 16, :].rearrange(
                        "p (bb b4 r u) -> p bb r b4 u", bb=2, b4=4, u=U
                    )
                    xt_h = xt[64 * p + 32 : 64 * p + 48, :].rearrange(
                        "p (bb b4 r u) -> p bb r b4 u", bb=2, b4=4, u=U
                    )
                    for bb in range(2):
                        B = 2 * p + bb
                        nc.vector.tensor_copy(
                            xt_c[:, bb],
                            c_red[32 * B : 32 * B + 16, :]
                            .rearrange("p (r b4) -> p r b4", b4=4)
                            .unsqueeze(3)
                            .broadcast_to((16, R, 4, U)),
                        )
                        nc.vector.tensor_copy(
                            xt_h[:, bb],
                            hT[32 * B : 32 * B + 16, :].rearrange(
                                "p (r b4 u) -> p r b4 u", b4=4, u=U
                            ),
                        )

            def mlp(g):
                xt = xts[g % 2]
                # ---- L1: full-K matmuls against zero-padded stationaries --
                # (the other pack's weight rows are zero, so each pack's
                # result is exact; full-array fp16 loads pipeline through
                # the background weight buffer, unlike row-tiled loads)
                a1s = [[None] * 4, [None] * 4]
                for s in range(4):
                    sblk = slice(s * 128, (s + 1) * 128)
                    for p in range(2):
                        ps1 = p1p.tile([128, TILE], F32, tag="ps1")
                        nc.tensor.matmul(
                            ps1[:],
                            w1x[p][:, sblk],
                            xt[:, :],
                            start=True, stop=True,
                        )
                        a1_s = acts.tile([128, TILE], F16, tag=f"a1{p}{s}")
                        # L1 bias folded in via the ones row -> plain relu
                        # (all-scalar during the 2-chunk ramp so the DVE
                        # queue drains the next gathers without stalling L1)
                        if s < 2 or g < 4:
                            nc.scalar.activation(a1_s[:], ps1[:], relu)
                        else:
                            nc.vector.tensor_scalar_max(a1_s[:], ps1[:], 0.0)
                        a1s[p][s] = a1_s

                # ---- L2: per output block t, both packs share PSUM --------
                # so the relu+bias activation runs 1024 wide and consecutive
                # matmuls reuse the same stationary weights.
                a2s = []
                for t in range(4):
                    ps2 = p2p.tile([128, 2, TILE], F32, tag="ps2")
                    for s in range(4):
                        for p in range(2):
                            nc.tensor.matmul(
                                ps2[:, p],
                                w2[:, s, t * 128 : (t + 1) * 128],
                                a1s[p][s][:],
                                start=(s == 0), stop=(s == 3),
                            )
                    a2_t = acts.tile([128, 2, TILE], F16, tag=f"a2{t}")
                    nc.scalar.activation(
                        a2_t[:], ps2[:], relu, bias=b2[:, t : t + 1],
                    )
                    a2s.append(a2_t)

                # ---- L3: a2 blocks stationary -> row-major out ------------
                # s-outer with a single start: the PSUM zero-region
                # pending-zero makes each q sub-tile's first write an
                # overwrite.
                ps3 = psm.tile([128, 2, 4, M], F32, tag="sm")
                for p in range(2):
                    for s in range(4):
                        for q in range(4):
                            nc.tensor.matmul(
                                ps3[:, p, q, :],
                                a2s[s][:, p, q * 128 : (q + 1) * 128],
                                w3[:, s, :],
                                start=(s == 0 and q == 0),
                                stop=(s == 3 and q == 3),
                            )
                return ps3

            def finish(g, ps3):
                # Emitted AFTER gather(g+1): the DVE b3-add depends on chunk
                # g's last L3 matmuls, and the DVE queue is FIFO - emitting
                # it before the next gather would head-block those copies
                # and stall chunk g+1's L1.
                for p in range(2):
                    o_tmp = outs.tile([128, 4, M], F32, tag="o_tmp")
                    nc.vector.scalar_tensor_tensor(
                        o_tmp[:], ps3[:, p], 1.0,
                        b3bc[:].unsqueeze(1).broadcast_to((128, 4, M)),
                        mybir.AluOpType.mult, op_add,
                    )
                    o_nat = outs.tile([128, 4, M], F16, tag="o_nat")
                    nc.scalar.activation(o_nat[:], o_tmp[:], tanh)
                    row0 = (2 * g + p) * TILE
                    nc.sync.dma_start(
                        out_rows[row0 : row0 + TILE].rearrange(
                            "(q p) m -> p q m", p=128
                        ),
                        o_nat[:],
                    )

            gather(0)
            for g in range(G):
                ps3 = mlp(g)
                if g + 1 < G:
                    gather(g + 1)
                finish(g, ps3)

    nc.compile()
    return nc


def _get_nc():
    if "nc" not in _CACHE:
        _CACHE["nc"] = _build()
    return _CACHE["nc"]


def _pack_weights(np_in):
    W1, b1 = np_in["W1"], np_in["b1"]
    W2, b2 = np_in["W2"], np_in["b2"]
    W3, b3 = np_in["W3"], np_in["b3"]
    w1p = np.zeros((128, H1), np.float32)
    for p in (0, 64):
        w1p[p + 0 : p + 16] = W1[16:32]     # c band
        w1p[p + 16] = b1                    # ones-row bias fold
        w1p[p + 32 : p + 48] = W1[32:48]    # h band
        w1p[p + 48 : p + 64] = W1[0:16]     # F band
    w2p = np.ascontiguousarray(
        W2.reshape(4, 128, H2).transpose(1, 0, 2), np.float16
    )
    w3p = np.ascontiguousarray(
        W3.reshape(4, 128, M).transpose(1, 0, 2), np.float16
    )
    b2p = np.ascontiguousarray(b2.reshape(4, 128).T, np.float32)
    b3bc = np.ascontiguousarray(np.broadcast_to(b3, (128, M)), np.float32)
    xinit = np.zeros((16, TILE), np.float16)
    xinit[0, :] = 1.0
    w1a = w1p.copy(); w1a[64:] = 0.0
    w1b = w1p.copy(); w1b[:64] = 0.0
    return {
        "w1a": w1a.astype(np.float16), "w1b": w1b.astype(np.float16),
        "w2p": w2p, "w3p": w3p, "b2p": b2p, "b3bc": b3bc, "xinit": xinit,
    }


def run(inputs, trace=False):
    nc = _get_nc()
    np_in = {k: np.ascontiguousarray(np.asarray(v, dtype=np.float32))
             for k, v in inputs.items()}
    packed = _pack_weights(np_in)
    in_maps = []
    for i in range(N_CORES):
        sl = slice(i * B_SH, (i + 1) * B_SH)
        m = {
            "C": np_in["C"][sl],
            "F": np_in["F"][sl],
            "H": np_in["H"][sl],
        }
        m.update(packed)
        in_maps.append(m)
    res = run_bass_kernel_spmd(nc, in_maps, list(range(N_CORES)), trace=trace)
    out = np.concatenate(
        [res.results[i]["out"].astype(np.float32) for i in range(N_CORES)], axis=0
    )
    return out, res


def kernel(**inputs):
    out, _ = run(inputs, trace=False)
    return out
